# revision 1
# baseline (speedup 1.0000x reference)
"""Trainium2 Bass kernel: multi-head causal attention with RoPE (LLaMA-style).

Problem: y = Attention(x) with B=2, S=2048, D=2048, H=16 heads, HD=128,
torch-Linear convention (y = x @ W.T), interleaved-rope, additive mask.

Sharding (8 NeuronCores): batch (2) x head-groups (4) grid.  Core c handles
batch b = c // 4 and heads 4g..4g+3 where g = c % 4 (tensor parallel:
wq/wk/wv column-parallel, wo row-parallel).  Each core returns a partial
y contribution [S, D]; the host sums the 4 partials per batch.

Layout strategy (no on-chip transposes anywhere):
  - Host pre-transposes: xT [D,S], wqT/wkT/wvT [D,E], woT [E,D].
  - Q^T,K^T computed directly in [hd, s] layout (hd = partitions) with the
    head-dim DEINTERLEAVED (rows 0-63 = even/"re" dims, 64-127 = odd/"im")
    by permuting wq/wk columns on the host; RoPE is then plain 64-partition
    elementwise ops.  The permutation is invisible to Q.K^T contraction.
  - scores are computed TRANSPOSED [sk, sq] so softmax-denominators come
    from a ones-matmul (column sums) and exp(scores)^T feeds the PV matmul
    directly as the moving operand: P^T never materializes.
  - attention out falls out as out^T [hd, sq] = exactly the stationary
    layout the wo row-parallel matmul wants.
Matmul inputs are bf16 (fp32 PSUM accumulation); softmax runs in fp32.
"""

import math
from contextlib import ExitStack

import numpy as np
import ml_dtypes

P = 128          # partitions / head dim
CW = 512         # s-chunk width (one PSUM bank of fp32)

_built_cache = {}


def _build(*, S, D, E, mask_mode):
    """Build + compile the SPMD Bass program for one core's shard.

    S: sequence length, D: model dim, E: head-columns per core (nH*128).
    mask_mode: 'causal' (use diag block + skip upper triangle),
               'none' (no mask, full attention),
               'generic' (arbitrary additive mask, applied everywhere).
    """
    import concourse.bacc as bacc
    import concourse.mybir as mybir
    import concourse.tile as tile

    f32 = mybir.dt.float32
    bf16 = mybir.dt.bfloat16
    Exp = mybir.ActivationFunctionType.Exp

    nDK = D // P       # k-tiles over model dim
    nH = E // P        # heads on this core
    nSC = S // CW      # 512-wide s-chunks
    nST = S // P       # 128-wide s-tiles
    TPC = CW // P      # s-tiles per chunk (4)
    SCALE = 1.0 / math.sqrt(P)
    causal = mask_mode == "causal"

    nc = bacc.Bacc("TRN2", target_bir_lowering=False, debug=False)

    xT = nc.dram_tensor("xT", [D, S], bf16, kind="ExternalInput").ap()
    wqT = nc.dram_tensor("wqT", [D, E], bf16, kind="ExternalInput").ap()
    wkT = nc.dram_tensor("wkT", [D, E], bf16, kind="ExternalInput").ap()
    wvT = nc.dram_tensor("wvT", [D, E], bf16, kind="ExternalInput").ap()
    woT = nc.dram_tensor("woT", [E, D], bf16, kind="ExternalInput").ap()
    cs = nc.dram_tensor("cs", [P, S], f32, kind="ExternalInput").ap()
    maskd = nc.dram_tensor("maskd", [P, P], bf16, kind="ExternalInput").ap()
    if mask_mode == "generic":
        maskT = nc.dram_tensor("maskT", [S, S], bf16, kind="ExternalInput").ap()
    y = nc.dram_tensor("y", [S, D], f32, kind="ExternalOutput").ap()

    with tile.TileContext(nc) as tc, ExitStack() as ctx:
        const = ctx.enter_context(tc.tile_pool(name="const", bufs=1))
        tp = ctx.enter_context(tc.tile_pool(name="tmp", bufs=2))
        expp = ctx.enter_context(tc.tile_pool(name="expp", bufs=8))
        sbB = ctx.enter_context(tc.tile_pool(name="sbB", bufs=2))
        yp = ctx.enter_context(tc.tile_pool(name="yp", bufs=3))
        psA = ctx.enter_context(tc.tile_pool(name="psA", bufs=5, space="PSUM"))
        psB = ctx.enter_context(tc.tile_pool(name="psB", bufs=2, space="PSUM"))
        psD = ctx.enter_context(tc.tile_pool(name="psD", bufs=1, space="PSUM"))

        # ---- persistent tiles --------------------------------------------
        qt = const.tile([P, nH, S], bf16)    # rotated Q^T  (re rows 0-63)
        kt = const.tile([P, nH, S], bf16)    # rotated K^T
        v = const.tile([P, nST, E], bf16)    # V [s within tile, stile, e]
        outT = const.tile([P, nH, S], bf16)  # attention out^T per head
        cs_t = const.tile([P, S], f32)       # rows 0-63 cos^T, 64-127 sin^T
        md = const.tile([P, P], bf16)        # exp(diag mask block^T): 0/1 for causal
        ones_col = const.tile([P, 1], bf16)

        nc.vector.memset(ones_col, 1.0)

        def rope(ps, dst, col):
            """ps: [128, CW] psum raw projection (re rows 0-63, im 64-127).
            dst: [128, CW] bf16 sbuf destination slice. col: s-slice.
            NB the walrus verifier requires all SBUF *inputs* of a
            tensor-tensor op to share a start partition; PSUM inputs and the
            output are exempt, so each product takes one aligned SBUF input
            and the combines read base-0 tiles."""
            re, im = ps[0:64, :], ps[64:128, :]
            cosv, sinv = cs_t[0:64, col], cs_t[64:128, col]
            t1 = tp.tile([64, CW], f32, tag="t1", name="t1")
            t2 = tp.tile([64, CW], f32, tag="t2", name="t2")
            nc.vector.tensor_mul(t1, re, cosv)
            nc.vector.tensor_mul(t2, im, sinv)
            nc.vector.tensor_sub(dst[0:64, :], t1, t2)
            t3 = tp.tile([64, CW], f32, tag="t1", name="t3")
            t4 = tp.tile([64, CW], f32, tag="t2", name="t4")
            nc.vector.tensor_mul(t3, re, sinv)
            nc.vector.tensor_mul(t4, im, cosv)
            nc.vector.tensor_add(dst[64:128, :], t3, t4)

        # ---- attention chunk emitter -------------------------------------
        # Masking happens OFF the PSUM critical chain: es := exp(scale*scores)
        # is multiplied by exp(mask) in SBUF (exact 0/1 for causal), so each
        # scores PSUM bank is held only for matmul -> exp.  scores+exp run
        # 3 iterations ahead of the denominator/PV matmuls to cover the
        # cross-engine exp latency.
        def attn_chunk(h, c, mk=None):
            qcol = slice(c * CW, (c + 1) * CW)
            jmax = TPC * c + TPC - 1 if causal else nST - 1
            ps_o = psB.tile([P, CW], f32, tag="psB", name="ps_o")
            ps_d = psD.tile([1, CW], f32, tag="psD", name="ps_d")
            ess = {}

            def emit_scores(j):
                o = max(0, j - TPC * c) * P if causal else 0
                ps_s = psA.tile([P, CW], f32, tag="psA", name="ps_s")
                nc.tensor.matmul(
                    ps_s[:, o:], kt[:, h, j * P:(j + 1) * P],
                    qt[:, h, c * CW + o:(c + 1) * CW],
                    start=True, stop=True)
                es = expp.tile([P, CW], bf16, tag="es", name="es")
                nc.scalar.activation(es[:, o:], ps_s[:, o:], Exp, scale=SCALE)
                if causal:
                    if j >= TPC * c:
                        nc.vector.tensor_mul(
                            es[:, o:o + P], es[:, o:o + P], md)
                elif mask_mode == "generic":
                    nc.vector.tensor_mul(es, es, mk[:, j, :])
                ess[j] = (es, o)

            for jj in range(min(3, jmax + 1)):
                emit_scores(jj)
            for j in range(jmax + 1):
                if j + 3 <= jmax:
                    emit_scores(j + 3)
                es, o = ess.pop(j)
                nc.tensor.matmul(ps_d[:, o:], ones_col, es[:, o:],
                                 start=(j == 0), stop=(j == jmax))
                nc.tensor.matmul(ps_o[:, o:], v[:, j, h * P:(h + 1) * P],
                                 es[:, o:], start=(j == 0), stop=(j == jmax))
            # Normalize out^T[:, sq] by 1/denom[sq].  Both PSUM banks are
            # freed by quick DVE copies; broadcast / reciprocal / multiply
            # run entirely in SBUF.
            dd = tp.tile([1, CW], f32, tag="rr", name="dd")
            nc.vector.tensor_copy(dd, ps_d)
            ou = expp.tile([P, CW], bf16, tag="ou", name="ou", bufs=4)
            nc.vector.tensor_copy(ou, ps_o)
            bc = sbB.tile([P, CW], f32, tag="bc", name="bc")
            nc.gpsimd.partition_broadcast(out_ap=bc, in_ap=dd)
            bcr = sbB.tile([P, CW], f32, tag="bcr", name="bcr")
            nc.vector.reciprocal_approx_fast(out=bcr, in_=bc)
            nc.vector.tensor_mul(outT[:, h, qcol], ou, bcr)

        # ---- output projection group (phase 3) ---------------------------
        nDC = D // CW

        def wo_group(m, wo_t):
            # h outer so each outT stationary serves nDC matmuls (one
            # LDWEIGHTS per head instead of per (head, chunk))
            pss = [psA.tile([P, CW], f32, tag="psA", name="ps_y")
                   for _ in range(nDC)]
            for h in range(nH):
                for dc in range(nDC):
                    nc.tensor.matmul(
                        pss[dc], outT[:, h, m * P:(m + 1) * P],
                        wo_t[:, h, dc * CW:(dc + 1) * CW],
                        start=(h == 0), stop=(h == nH - 1))
            for dc in range(nDC):
                yo = yp.tile([P, CW], f32, tag="yo", name="yo")
                nc.vector.tensor_copy(yo, pss[dc])
                nc.sync.dma_start(
                    out=y[m * P:(m + 1) * P, dc * CW:(dc + 1) * CW], in_=yo)

        # ---- projections + attention, engine-interleaved -----------------
        # Emission order matters: engines execute their queues in program
        # order, so attention for head h (ACT-heavy exp chain, DVE tails) is
        # emitted interleaved with head h+1's K/Q projection groups (pure PE
        # work), and the last head's attention with the wo matmul groups.
        with tc.tile_pool(name="xw", bufs=1) as xtp, \
             tc.tile_pool(name="wz", bufs=4) as wpool:
            nXQ = min(8, nDK)
            nKQ = nDK // nXQ
            NH2 = nDK // 2
            wdmas = {"q": wqT.rearrange("(dk p) e -> p dk e", p=P),
                     "k": wkT.rearrange("(dk p) e -> p dk e", p=P),
                     "v": wvT.rearrange("(dk p) e -> p dk e", p=P)}

            def load_w(key):
                wts = []
                for kh in range(2):
                    wt = wpool.tile([P, NH2, E], bf16, tag="w", name="wt")
                    nc.sync.dma_start(
                        out=wt, in_=wdmas[key][:, kh * NH2:(kh + 1) * NH2, :])
                    wts.append(wt)
                return wts

            # DMA order: wv first (V projection runs first) on the sync ring,
            # x^T dk-eighths on the scalar ring, then wk/cs/md/wq.
            wv_t = load_w("v")
            xts = []
            for kq in range(nXQ):
                xt = xtp.tile([P, nKQ, S], bf16, tag=f"xt{kq}", name="xt")
                nc.scalar.dma_start(
                    out=xt,
                    in_=xT.rearrange("(dk p) s -> p dk s", p=P)[
                        :, kq * nKQ:(kq + 1) * nKQ, :])
                xts.append(xt)
            wk_t = load_w("k")
            nc.sync.dma_start(out=cs_t, in_=cs)
            nc.sync.dma_start(out=md, in_=maskd)
            wq_t = load_w("q")

            def xslice(dk, ssl):
                return xts[dk // nKQ][:, dk % nKQ, ssl]

            def wslice(wts, dk, esl):
                return wts[dk // NH2][:, dk % NH2, esl]

            # V projection (all heads at once: rhs = all E columns)
            for st in range(nST):
                ssl = slice(st * P, (st + 1) * P)
                ps = psA.tile([P, CW], f32, tag="psA", name="ps_v")
                for dk in range(nDK):
                    nc.tensor.matmul(
                        ps[:, 0:E], xslice(dk, ssl), wslice(wv_t, dk, slice(0, E)),
                        start=(dk == 0), stop=(dk == nDK - 1))
                nc.scalar.copy(v[:, st, :], ps[:, 0:E])

            def qk_group(wts, dest, h, sc):
                esl = slice(h * P, (h + 1) * P)
                col = slice(sc * CW, (sc + 1) * CW)
                ps = psA.tile([P, CW], f32, tag="psA", name="ps_qk")
                for dk in range(nDK):
                    nc.tensor.matmul(
                        ps, wslice(wts, dk, esl), xslice(dk, col),
                        start=(dk == 0), stop=(dk == nDK - 1))
                rope(ps, dest[:, h, col], col)

            for sc in range(nSC):
                qk_group(wk_t, kt, 0, sc)
            for sc in range(nSC):
                qk_group(wq_t, qt, 0, sc)

            if causal:
                # attention for head h paced against head h+1's K then Q
                # projection groups (2 per chunk): the PE-only projection
                # work absorbs the ACT-side exp latency of the attention
                # chain, and only head 0's projections sit ahead of the
                # first attention chunk
                for h in range(nH):
                    units = ([(wk_t, kt, sc) for sc in range(nSC)] +
                             [(wq_t, qt, sc) for sc in range(nSC)])
                    for c in range(nSC):
                        attn_chunk(h, c)
                        if h + 1 < nH:
                            for u in range(2 * c, min(2 * c + 2, len(units))):
                                wts, dest, sc_ = units[u]
                                qk_group(wts, dest, h + 1, sc_)
            else:
                for h in range(1, nH):
                    for sc in range(nSC):
                        qk_group(wk_t, kt, h, sc)
                        qk_group(wq_t, qt, h, sc)

        # ---- late pool (reuses xt/w space) -------------------------------
        late = ctx.enter_context(tc.tile_pool(name="late", bufs=1))
        wo_t = late.tile([P, nH, D], bf16)
        nc.sync.dma_start(out=wo_t, in_=woT.rearrange("(h p) d -> p h d", p=P))

        if not causal:
            for c in range(nSC):
                mk = None
                if mask_mode == "generic":
                    mk = late.tile([P, nST, CW], bf16, tag="mk", name="mk",
                                   bufs=2)
                    nc.sync.dma_start(
                        out=mk,
                        in_=maskT.rearrange("(j p) q -> p j q", p=P)[
                            :, :, c * CW:(c + 1) * CW])
                for h in range(nH):
                    attn_chunk(h, c, mk=mk)
        for m in range(nST):
            wo_group(m, wo_t)

    nc.compile()
    return nc


def _get_built(mask_mode, S, D, E):
    key = (mask_mode, S, D, E)
    if key not in _built_cache:
        _built_cache[key] = _build(S=S, D=D, E=E, mask_mode=mask_mode)
    return _built_cache[key]


def _classify_mask(mask):
    S = mask.shape[0]
    if not mask.any():
        return "none"
    causal = np.where(np.triu(np.ones((S, S), dtype=bool), k=1),
                      np.float32(-1e9), np.float32(0.0))
    if np.array_equal(mask, causal):
        return "causal"
    return "generic"


def make_in_maps(x, wq, wk, wv, wo, freqs_cos, freqs_sin, mask, n_cores=8):
    """Host-side sharding + layout prep. Returns (in_maps, mask_mode, meta)."""
    bf = ml_dtypes.bfloat16
    x = np.asarray(x, np.float32)
    B, S, D = x.shape
    groups = n_cores // B
    E = D // groups
    nH = E // P
    scale = 1.0 / math.sqrt(P)

    mask = np.asarray(mask, np.float32)
    mode = _classify_mask(mask)

    fc = np.asarray(freqs_cos, np.float32)
    fs = np.asarray(freqs_sin, np.float32)
    cs = np.concatenate(
        [np.ascontiguousarray(fc.T), np.ascontiguousarray(fs.T)], axis=0
    ).astype(np.float32)                      # [128, S]
    # masking is applied multiplicatively on exp(scores): exp(mask) — exact
    # 0/1 for the causal -1e9/0 mask
    maskd = np.exp(np.ascontiguousarray(mask[0:P, 0:P].T)).astype(bf)

    # per-head deinterleave: head-local columns [0,2,...,126,1,3,...,127]
    perm1 = np.concatenate([np.arange(0, P, 2), np.arange(1, P, 2)])
    permE = np.concatenate([h * P + perm1 for h in range(nH)])

    wqT_f = np.asarray(wq, np.float32).T      # [D, D]
    wkT_f = np.asarray(wk, np.float32).T
    wvT_f = np.asarray(wv, np.float32).T
    woT_f = np.asarray(wo, np.float32).T      # [E_total, D]

    if mode == "generic":
        maskT_bf = np.exp(np.ascontiguousarray(mask.T)).astype(bf)

    xT_b = [np.ascontiguousarray(x[b].T).astype(bf) for b in range(B)]

    in_maps = []
    for c in range(n_cores):
        b, g = divmod(c, groups)
        es = slice(g * E, (g + 1) * E)
        m = {
            "xT": xT_b[b],
            "wqT": np.ascontiguousarray(wqT_f[:, es][:, permE]).astype(bf),
            "wkT": np.ascontiguousarray(wkT_f[:, es][:, permE]).astype(bf),
            "wvT": np.ascontiguousarray(wvT_f[:, es]).astype(bf),
            "woT": np.ascontiguousarray(woT_f[es, :]).astype(bf),
            "cs": cs,
            "maskd": maskd,
        }
        if mode == "generic":
            m["maskT"] = maskT_bf
        in_maps.append(m)
    return in_maps, mode, (B, S, D, E, groups)


def kernel(x, wq, wk, wv, wo, freqs_cos, freqs_sin, mask, start_pos=0, **_):
    from concourse.bass_utils import run_bass_kernel_spmd

    in_maps, mode, (B, S, D, E, groups) = make_in_maps(
        x, wq, wk, wv, wo, freqs_cos, freqs_sin, mask)
    nc = _get_built(mode, S, D, E)
    res = run_bass_kernel_spmd(nc, in_maps, core_ids=list(range(len(in_maps))))
    parts = [r["y"] for r in res.results]
    out = np.stack(
        [np.sum(parts[b * groups:(b + 1) * groups], axis=0) for b in range(B)]
    ).astype(np.float32)
    return out



# revision 4
# speedup vs baseline: 1.0964x; 1.0964x over previous
"""Trainium2 Bass kernel: multi-head causal attention with RoPE (LLaMA-style).

Problem: y = Attention(x) with B=2, S=2048, D=2048, H=16 heads, HD=128,
torch-Linear convention (y = x @ W.T), interleaved-rope, additive mask.

Sharding (8 NeuronCores): batch (2) x head-groups (4) grid.  Core c handles
batch b = c // 4 and heads 4g..4g+3 where g = c % 4 (tensor parallel:
wq/wk/wv column-parallel, wo row-parallel).  Each core returns a partial
y contribution [S, D]; the host sums the 4 partials per batch.

Layout strategy (no on-chip transposes anywhere):
  - Host pre-transposes AND pre-swizzles DMA layouts: x as 4 s-major chunks
    [P, nDK, 512] (so the V projection can start after 1/4 of x lands),
    wq/wk/wv as [P, nDK, E], wo as [P, nH, D] -- every DMA is a contiguous
    per-partition run.
  - Q^T,K^T computed directly in [hd, s] layout (hd = partitions) with the
    head-dim DEINTERLEAVED (rows 0-63 = even/"re" dims, 64-127 = odd/"im")
    by permuting wq/wk columns on the host; RoPE is then 5 DVE ops per
    chunk (products exploit the walrus PSUM-input exemption; combines run
    all-bf16 at 2x DVE rate).
  - scores are computed TRANSPOSED [sk, sq] so softmax-denominators come
    from a ones-matmul (column sums) and exp(scores)^T feeds the PV matmul
    directly as the moving operand: P^T never materializes.
  - causal mask applied INSIDE PSUM by an extra accumulating matmul
    (identity stationary x (-1e9 triangle) moving) so exp sees masked
    scores directly -- no DVE op or cross-engine hop on the es path.
  - attention out falls out as out^T [hd, sq] = exactly the stationary
    layout the wo row-parallel matmul wants.  wo phase double-buffers two
    4-bank PSUM sets, splits PSUM->SBUF copies between ACT and DVE, and
    ships y as 1 MB per-row-tile DMAs.
  - PE is warmed with identity matmuls during the initial DMA wait (HAM
    clock-gate ramps after ~3.4us of activity).
Matmul inputs are bf16 (fp32 PSUM accumulation); softmax runs in fp32.
"""

import math
from contextlib import ExitStack

import numpy as np
import ml_dtypes

P = 128          # partitions / head dim
CW = 512         # s-chunk width (one PSUM bank of fp32)

_built_cache = {}


def _build(*, S, D, E, mask_mode):
    """Build + compile the SPMD Bass program for one core's shard.

    S: sequence length, D: model dim, E: head-columns per core (nH*128).
    mask_mode: 'causal' (use diag mask-add matmul + skip upper triangle),
               'none' (no mask, full attention),
               'generic' (arbitrary additive mask, applied everywhere).
    """
    import concourse.bacc as bacc
    import concourse.mybir as mybir
    import concourse.tile as tile

    f32 = mybir.dt.float32
    bf16 = mybir.dt.bfloat16
    Exp = mybir.ActivationFunctionType.Exp

    nDK = D // P       # k-tiles over model dim
    nH = E // P        # heads on this core
    nSC = S // CW      # 512-wide s-chunks
    nST = S // P       # 128-wide s-tiles
    TPC = CW // P      # s-tiles per chunk (4)
    NXC = S // CW      # x DMA chunks along s (== nSC)
    SCALE = 1.0 / math.sqrt(P)
    causal = mask_mode == "causal"

    nc = bacc.Bacc("TRN2", target_bir_lowering=False, debug=False)

    # host pre-swizzled layouts (all contiguous per-partition runs)
    xr = nc.dram_tensor("xr", [NXC, P, nDK, CW], bf16, kind="ExternalInput").ap()
    wqr = nc.dram_tensor("wqr", [P, nDK, E], bf16, kind="ExternalInput").ap()
    wkr = nc.dram_tensor("wkr", [P, nDK, E], bf16, kind="ExternalInput").ap()
    wvr = nc.dram_tensor("wvr", [P, nDK, E], bf16, kind="ExternalInput").ap()
    wor = nc.dram_tensor("wor", [P, nH, D], bf16, kind="ExternalInput").ap()
    cs = nc.dram_tensor("cs", [P, S], bf16, kind="ExternalInput").ap()   # [cos;sin]
    cc = nc.dram_tensor("cc", [P, S], bf16, kind="ExternalInput").ap()   # [cos;cos]
    identd = nc.dram_tensor("identd", [P, P], bf16, kind="ExternalInput").ap()
    maskd = nc.dram_tensor("maskd", [P, P], bf16, kind="ExternalInput").ap()
    if mask_mode == "generic":
        maskT = nc.dram_tensor("maskT", [S, S], bf16, kind="ExternalInput").ap()
    y = nc.dram_tensor("y", [S, D], f32, kind="ExternalOutput").ap()

    with tile.TileContext(nc) as tc, ExitStack() as ctx:
        const = ctx.enter_context(tc.tile_pool(name="const", bufs=1))
        tp = ctx.enter_context(tc.tile_pool(name="tmp", bufs=2))
        expp = ctx.enter_context(tc.tile_pool(name="expp", bufs=8))
        sbB = ctx.enter_context(tc.tile_pool(name="sbB", bufs=2))
        psA = ctx.enter_context(tc.tile_pool(name="psA", bufs=5, space="PSUM"))
        psB = ctx.enter_context(tc.tile_pool(name="psB", bufs=2, space="PSUM"))
        psD = ctx.enter_context(tc.tile_pool(name="psD", bufs=1, space="PSUM"))

        # ---- persistent tiles --------------------------------------------
        qt = const.tile([P, nH, S], bf16)    # rotated Q^T  (re rows 0-63)
        kt = const.tile([P, nH, S], bf16)    # rotated K^T
        v = const.tile([P, nST, E], bf16)    # V [s within tile, stile, e]
        outT = const.tile([P, nH, S], bf16)  # attention out^T per head
        cs_t = const.tile([P, S], bf16)      # rows 0-63 cos^T, 64-127 sin^T
        cc_t = const.tile([P, S], bf16)      # rows 0-63 AND 64-127 cos^T
        ident = const.tile([P, P], bf16)     # identity (warmup + mask-add)
        md = const.tile([P, P], bf16)        # causal: -1e9 strict lower tri
        ones_col = const.tile([P, 1], bf16)

        nc.vector.memset(ones_col, 1.0)

        # ---- warmup: identity matmuls during the initial DMA wait --------
        # (HAM un-throttles after ~3.4us of PE activity; also preload the
        # ACT exp table so the first attention chunk doesn't pay ~2.7us)
        nc.sync.dma_start(out=ident, in_=identd)
        nc.sync.dma_start(out=md, in_=maskd)
        wexp = tp.tile([P, 1], f32, tag="rr", name="wexp")
        nc.scalar.activation(wexp, ones_col, Exp, scale=1.0)
        warm = psB.tile([P, P], f32, tag="psB", name="warm")
        for _ in range(10):
            nc.tensor.matmul(warm, ident, ident, start=True, stop=True)

        def rope(ps, dst, col):
            """ps: [128, CW] psum raw projection (re rows 0-63, im 64-127).
            dst: [128, CW] bf16 sbuf destination slice. col: s-slice.
            5 DVE ops: PSUM inputs are exempt from the walrus shared-start-
            partition rule, so the three products each use ONE aligned SBUF
            input; the two combines are all-bf16 (2x DVE rate)."""
            rB = tp.tile([P, CW], bf16, tag="rB", name="rB")
            nc.vector.tensor_mul(rB, ps, cc_t[:, col])              # re*c|im*c
            rA = tp.tile([P, CW], bf16, tag="rA", name="rA")
            nc.vector.tensor_mul(rA[0:64], ps[64:128], cs_t[64:128, col])
            nc.vector.tensor_mul(rA[64:128], ps[0:64], cs_t[64:128, col])
            nc.vector.tensor_sub(dst[0:64], rB[0:64], rA[0:64])
            nc.vector.tensor_add(dst[64:128], rA[64:128], rB[64:128])

        # ---- attention chunk emitter -------------------------------------
        # The causal mask is accumulated INTO the scores PSUM bank by a
        # second matmul (ident stationary, -1e9-triangle moving), so
        # exp(scale*psum) is immediately the masked es.  scores+exp run
        # 4 iterations ahead of the denominator/PV matmuls to cover the
        # cross-engine exp latency.
        def attn_chunk(h, c, mk=None):
            qcol = slice(c * CW, (c + 1) * CW)
            jmax = TPC * c + TPC - 1 if causal else nST - 1
            ps_o = psB.tile([P, CW], f32, tag="psB", name="ps_o")
            ps_d = psD.tile([1, CW], f32, tag="psD", name="ps_d")
            ess = {}
            AHEAD = 4

            def emit_scores(j):
                o = max(0, j - TPC * c) * P if causal else 0
                diag = causal and j >= TPC * c
                ps_s = psA.tile([P, CW], f32, tag="psA", name="ps_s")
                nc.tensor.matmul(
                    ps_s[:, o:], kt[:, h, j * P:(j + 1) * P],
                    qt[:, h, c * CW + o:(c + 1) * CW],
                    start=True, stop=not diag, skip_group_check=True)
                if diag:
                    nc.tensor.matmul(
                        ps_s[:, o:o + P], ident, md,
                        start=False, stop=True, skip_group_check=True)
                es = expp.tile([P, CW], bf16, tag="es", name="es")
                nc.scalar.activation(es[:, o:], ps_s[:, o:], Exp, scale=SCALE)
                if mask_mode == "generic":
                    nc.vector.tensor_mul(es, es, mk[:, j, :])
                ess[j] = (es, o)

            for jj in range(min(AHEAD, jmax + 1)):
                emit_scores(jj)
            for j in range(jmax + 1):
                if j + AHEAD <= jmax:
                    emit_scores(j + AHEAD)
                es, o = ess.pop(j)
                nc.tensor.matmul(ps_d[:, o:], ones_col, es[:, o:],
                                 start=(j == 0), stop=(j == jmax))
                nc.tensor.matmul(ps_o[:, o:], v[:, j, h * P:(h + 1) * P],
                                 es[:, o:], start=(j == 0), stop=(j == jmax))
            # Normalize out^T[:, sq] by 1/denom[sq].  Both PSUM banks are
            # freed by ACT copies (DVE is the busy engine in this phase);
            # broadcast / reciprocal / multiply run entirely in SBUF.
            dd = tp.tile([1, CW], f32, tag="rr", name="dd")
            nc.scalar.copy(dd, ps_d)
            ou = expp.tile([P, CW], bf16, tag="ou", name="ou", bufs=4)
            nc.scalar.copy(ou, ps_o)
            bc = sbB.tile([P, CW], f32, tag="bc", name="bc")
            nc.gpsimd.partition_broadcast(out_ap=bc, in_ap=dd)
            bcr = sbB.tile([P, CW], f32, tag="bcr", name="bcr")
            nc.vector.reciprocal_approx_fast(out=bcr, in_=bc)
            nc.vector.tensor_mul(outT[:, h, qcol], ou, bcr)

        # ---- projections + attention, engine-interleaved -----------------
        # Emission order matters: engines execute their queues in program
        # order, so attention for head h (ACT-heavy exp chain, DVE tails) is
        # emitted interleaved with head h+1's K/Q projection groups (pure PE
        # work), and the last head's attention with the wo matmul groups.
        with tc.tile_pool(name="xw", bufs=1) as xtp, \
             tc.tile_pool(name="wz", bufs=4) as wpool:
            NH2 = nDK // 2

            def load_w(key, src):
                wts = []
                for kh in range(2):
                    wt = wpool.tile([P, NH2, E], bf16, tag="w", name="wt")
                    nc.sync.dma_start(
                        out=wt, in_=src[:, kh * NH2:(kh + 1) * NH2, :])
                    wts.append(wt)
                return wts

            # DMA order: wv first (V projection runs first) on the sync
            # ring; x^T s-chunks stream on the scalar ring in parallel.
            wv_t = load_w("v", wvr)
            xts = []
            for kq in range(NXC):
                xt = xtp.tile([P, nDK, CW], bf16, tag=f"xt{kq}", name="xt")
                nc.scalar.dma_start(out=xt, in_=xr[kq])
                xts.append(xt)
            wk_t = load_w("k", wkr)
            nc.sync.dma_start(out=cs_t, in_=cs)
            nc.sync.dma_start(out=cc_t, in_=cc)
            wq_t = load_w("q", wqr)

            def xslice(dk, ssl):
                # ssl is a global s-slice fully inside one 512-wide chunk
                lo = ssl.start
                return xts[lo // CW][:, dk, lo % CW:lo % CW + (ssl.stop - lo)]

            def wslice(wts, dk, esl):
                return wts[dk // NH2][:, dk % NH2, esl]

            # V projection (all heads at once: rhs = all E columns);
            # s-tile st consumes x chunk st//TPC only.
            for st in range(nST):
                ssl = slice(st * P, (st + 1) * P)
                ps = psA.tile([P, CW], f32, tag="psA", name="ps_v")
                for dk in range(nDK):
                    nc.tensor.matmul(
                        ps[:, 0:E], xslice(dk, ssl), wslice(wv_t, dk, slice(0, E)),
                        start=(dk == 0), stop=(dk == nDK - 1))
                nc.scalar.copy(v[:, st, :], ps[:, 0:E])

            def qk_group(wts, dest, h, sc):
                esl = slice(h * P, (h + 1) * P)
                col = slice(sc * CW, (sc + 1) * CW)
                ps = psA.tile([P, CW], f32, tag="psA", name="ps_qk")
                for dk in range(nDK):
                    nc.tensor.matmul(
                        ps, wslice(wts, dk, esl), xslice(dk, col),
                        start=(dk == 0), stop=(dk == nDK - 1))
                rope(ps, dest[:, h, col], col)

            for sc in range(nSC):
                qk_group(wk_t, kt, 0, sc)
            for sc in range(nSC):
                qk_group(wq_t, qt, 0, sc)

            if causal:
                # attention for head h paced against head h+1's K then Q
                # projection groups (2 per chunk): the PE-only projection
                # work absorbs the ACT-side exp latency of the attention
                # chain, and only head 0's projections sit ahead of the
                # first attention chunk
                for h in range(nH):
                    units = ([(wk_t, kt, sc) for sc in range(nSC)] +
                             [(wq_t, qt, sc) for sc in range(nSC)])
                    for c in range(nSC):
                        attn_chunk(h, c)
                        if h + 1 < nH:
                            for u in range(2 * c, min(2 * c + 2, len(units))):
                                wts, dest, sc_ = units[u]
                                qk_group(wts, dest, h + 1, sc_)
            else:
                for h in range(1, nH):
                    for sc in range(nSC):
                        qk_group(wk_t, kt, h, sc)
                        qk_group(wq_t, qt, h, sc)

        # ---- late pool (reuses xt/w space) -------------------------------
        late = ctx.enter_context(tc.tile_pool(name="late", bufs=1))
        wo_t = late.tile([P, nH, D], bf16)
        nc.sync.dma_start(out=wo_t, in_=wor)

        if not causal:
            for c in range(nSC):
                mk = None
                if mask_mode == "generic":
                    mk = late.tile([P, nST, CW], bf16, tag="mk", name="mk",
                                   bufs=2)
                    nc.sync.dma_start(
                        out=mk,
                        in_=maskT.rearrange("(j p) q -> p j q", p=P)[
                            :, :, c * CW:(c + 1) * CW])
                for h in range(nH):
                    attn_chunk(h, c, mk=mk)

        # ---- output projection (phase 3) ---------------------------------
        # Two alternating 4-bank PSUM sets so group m+1's matmuls overlap
        # group m's copies; copies split ACT/DVE; one 1 MB y-DMA per m.
        nDC = D // CW

        def wo_group(m):
            if m % 2 == 0:
                pss = [psA.tile([P, CW], f32, tag="psA", name="ps_y")
                       for _ in range(nDC)]
            else:
                pss = [psB.tile([P, CW], f32, tag="psB", name="ps_y"),
                       psB.tile([P, CW], f32, tag="psB", name="ps_y"),
                       psD.tile([P, CW], f32, tag="psD", name="ps_y"),
                       psA.tile([P, CW], f32, tag="psA", name="ps_y")]
            for h in range(nH):
                for dc in range(nDC):
                    nc.tensor.matmul(
                        pss[dc], outT[:, h, m * P:(m + 1) * P],
                        wo_t[:, h, dc * CW:(dc + 1) * CW],
                        start=(h == 0), stop=(h == nH - 1))
            yo = late.tile([P, D], f32, tag="yo", name="yo", bufs=2)
            for dc in range(nDC):
                dst = yo[:, dc * CW:(dc + 1) * CW]
                if dc % 2 == 0:
                    nc.scalar.copy(dst, pss[dc])
                else:
                    nc.vector.tensor_copy(dst, pss[dc])
            nc.sync.dma_start(out=y[m * P:(m + 1) * P, :], in_=yo)

        for m in range(nST):
            wo_group(m)

    nc.compile()
    return nc


def _get_built(mask_mode, S, D, E):
    key = (mask_mode, S, D, E)
    if key not in _built_cache:
        _built_cache[key] = _build(S=S, D=D, E=E, mask_mode=mask_mode)
    return _built_cache[key]


def _classify_mask(mask):
    S = mask.shape[0]
    if not mask.any():
        return "none"
    causal = np.where(np.triu(np.ones((S, S), dtype=bool), k=1),
                      np.float32(-1e9), np.float32(0.0))
    if np.array_equal(mask, causal):
        return "causal"
    return "generic"


def make_in_maps(x, wq, wk, wv, wo, freqs_cos, freqs_sin, mask, n_cores=8):
    """Host-side sharding + layout prep. Returns (in_maps, mask_mode, meta)."""
    bf = ml_dtypes.bfloat16
    x = np.asarray(x, np.float32)
    B, S, D = x.shape
    groups = n_cores // B
    E = D // groups
    nH = E // P
    nDK = D // P
    NXC = S // CW

    mask = np.asarray(mask, np.float32)
    mode = _classify_mask(mask)

    fc = np.asarray(freqs_cos, np.float32)
    fs = np.asarray(freqs_sin, np.float32)
    cs = np.concatenate(
        [np.ascontiguousarray(fc.T), np.ascontiguousarray(fs.T)], axis=0
    ).astype(bf)                              # [128, S]: [cos; sin]
    cc = np.concatenate(
        [np.ascontiguousarray(fc.T), np.ascontiguousarray(fc.T)], axis=0
    ).astype(bf)                              # [128, S]: [cos; cos]

    identd = np.eye(P, dtype=np.float32).astype(bf)
    if mode == "causal":
        # strict lower triangle (sk > sq) gets -1e9: added into scores PSUM
        maskd = np.where(np.arange(P)[:, None] > np.arange(P)[None, :],
                         np.float32(-1e9), np.float32(0.0)).astype(bf)
    else:
        maskd = np.zeros((P, P), np.float32).astype(bf)

    # per-head deinterleave: head-local columns [0,2,...,126,1,3,...,127]
    perm1 = np.concatenate([np.arange(0, P, 2), np.arange(1, P, 2)])
    permE = np.concatenate([h * P + perm1 for h in range(nH)])

    wqT_f = np.asarray(wq, np.float32).T      # [D, D]
    wkT_f = np.asarray(wk, np.float32).T
    wvT_f = np.asarray(wv, np.float32).T
    woT_f = np.asarray(wo, np.float32).T      # [E_total, D]

    if mode == "generic":
        maskT_bf = np.exp(np.ascontiguousarray(mask.T)).astype(bf)

    def swz_w(wT):   # [D, E] -> [P, nDK, E]
        return np.ascontiguousarray(
            wT.reshape(nDK, P, E).transpose(1, 0, 2)).astype(bf)

    def swz_x(xT):   # [D, S] -> [NXC, P, nDK, CW]
        return np.ascontiguousarray(
            xT.reshape(nDK, P, NXC, CW).transpose(2, 1, 0, 3)).astype(bf)

    xr_b = [swz_x(x[b].T) for b in range(B)]

    in_maps = []
    for c in range(n_cores):
        b, g = divmod(c, groups)
        es = slice(g * E, (g + 1) * E)
        woT_g = woT_f[es, :]                  # [E, D]
        m = {
            "xr": xr_b[b],
            "wqr": swz_w(wqT_f[:, es][:, permE]),
            "wkr": swz_w(wkT_f[:, es][:, permE]),
            "wvr": swz_w(wvT_f[:, es]),
            "wor": np.ascontiguousarray(
                woT_g.reshape(nH, P, D).transpose(1, 0, 2)).astype(bf),
            "cs": cs,
            "cc": cc,
            "identd": identd,
            "maskd": maskd,
        }
        if mode == "generic":
            m["maskT"] = maskT_bf
        in_maps.append(m)
    return in_maps, mode, (B, S, D, E, groups)


def kernel(x, wq, wk, wv, wo, freqs_cos, freqs_sin, mask, start_pos=0, **_):
    from concourse.bass_utils import run_bass_kernel_spmd

    in_maps, mode, (B, S, D, E, groups) = make_in_maps(
        x, wq, wk, wv, wo, freqs_cos, freqs_sin, mask)
    nc = _get_built(mode, S, D, E)
    res = run_bass_kernel_spmd(nc, in_maps, core_ids=list(range(len(in_maps))))
    parts = [r["y"] for r in res.results]
    out = np.stack(
        [np.sum(parts[b * groups:(b + 1) * groups], axis=0) for b in range(B)]
    ).astype(np.float32)
    return out


# revision 15
# speedup vs baseline: 1.1007x; 1.0039x over previous
"""Trainium2 Bass kernel: multi-head causal attention with RoPE (LLaMA-style).

Problem: y = Attention(x) with B=2, S=2048, D=2048, H=16 heads, HD=128,
torch-Linear convention (y = x @ W.T), interleaved-rope, additive mask.

Sharding (8 NeuronCores): batch (2) x head-groups (4) grid.  Core c handles
batch b = c // 4 and heads 4g..4g+3 where g = c % 4 (tensor parallel:
wq/wk/wv column-parallel, wo row-parallel).  Each core returns a partial
y contribution [S, D]; the host sums the 4 partials per batch.

Layout strategy (no on-chip transposes anywhere):
  - Host pre-transposes AND pre-swizzles DMA layouts: x as 4 s-major chunks
    [P, nDK, 512] (so the V projection can start after 1/4 of x lands),
    wq/wk/wv as [P, nDK, E], wo as [P, nH, D] -- every DMA is a contiguous
    per-partition run.
  - Q^T,K^T computed directly in [hd, s] layout (hd = partitions) with the
    head-dim DEINTERLEAVED (rows 0-63 = even/"re" dims, 64-127 = odd/"im")
    by permuting wq/wk columns on the host; RoPE is then 5 DVE ops per
    chunk (products exploit the walrus PSUM-input exemption; combines run
    all-bf16 at 2x DVE rate).
  - scores are computed TRANSPOSED [sk, sq] so softmax-denominators come
    from a ones-matmul (column sums) and exp(scores)^T feeds the PV matmul
    directly as the moving operand: P^T never materializes.
  - causal mask applied INSIDE PSUM by an extra accumulating matmul
    (identity stationary x (-1e9 triangle) moving) so exp sees masked
    scores directly -- no DVE op or cross-engine hop on the es path.
  - attention out falls out as out^T [hd, sq] = exactly the stationary
    layout the wo row-parallel matmul wants.  wo phase double-buffers two
    4-bank PSUM sets, splits PSUM->SBUF copies between ACT and DVE, and
    ships y as 1 MB per-row-tile DMAs.
  - PE is warmed with identity matmuls during the initial DMA wait (HAM
    clock-gate ramps after ~3.4us of activity).
Matmul inputs are bf16 (fp32 PSUM accumulation); softmax runs in fp32.
"""

import math
from contextlib import ExitStack
from itertools import chain

import numpy as np
import ml_dtypes

P = 128          # partitions / head dim
CW = 512         # s-chunk width (one PSUM bank of fp32)
NXC = 8          # x DMA chunks along s

_built_cache = {}


def _build(*, S, D, E, mask_mode):
    """Build + compile the SPMD Bass program for one core's shard.

    S: sequence length, D: model dim, E: head-columns per core (nH*128).
    mask_mode: 'causal' (use diag mask-add matmul + skip upper triangle),
               'none' (no mask, full attention),
               'generic' (arbitrary additive mask, applied everywhere).
    """
    import concourse.bacc as bacc
    import concourse.mybir as mybir
    import concourse.tile as tile

    f32 = mybir.dt.float32
    bf16 = mybir.dt.bfloat16
    Exp = mybir.ActivationFunctionType.Exp

    nDK = D // P       # k-tiles over model dim
    nH = E // P        # heads on this core
    nSC = S // CW      # 512-wide s-chunks
    nST = S // P       # 128-wide s-tiles
    TPC = CW // P      # s-tiles per chunk (4)
    SCW = S // NXC     # x chunk width (256)
    SCALE = 1.0 / math.sqrt(P)
    causal = mask_mode == "causal"

    nc = bacc.Bacc("TRN2", target_bir_lowering=False, debug=False)

    # host pre-swizzled layouts (all contiguous per-partition runs)
    xr = nc.dram_tensor("xr", [NXC, P, nDK, SCW], bf16, kind="ExternalInput").ap()
    wqr = nc.dram_tensor("wqr", [P, nDK, E], bf16, kind="ExternalInput").ap()
    wkr = nc.dram_tensor("wkr", [P, nDK, E], bf16, kind="ExternalInput").ap()
    wvr = nc.dram_tensor("wvr", [P, nDK, E], bf16, kind="ExternalInput").ap()
    wor = nc.dram_tensor("wor", [P, nH, D], bf16, kind="ExternalInput").ap()
    cs = nc.dram_tensor("cs", [P, S], bf16, kind="ExternalInput").ap()   # [cos;sin]
    cc = nc.dram_tensor("cc", [P, S], bf16, kind="ExternalInput").ap()   # [cos;cos]
    identd = nc.dram_tensor("identd", [P, P], bf16, kind="ExternalInput").ap()
    maskd = nc.dram_tensor("maskd", [P, P], bf16, kind="ExternalInput").ap()
    if mask_mode == "generic":
        maskT = nc.dram_tensor("maskT", [S, S], bf16, kind="ExternalInput").ap()
    y = nc.dram_tensor("y", [S, D], bf16, kind="ExternalOutput").ap()

    with tile.TileContext(nc) as tc, ExitStack() as ctx:
        const = ctx.enter_context(tc.tile_pool(name="const", bufs=1))
        tp = ctx.enter_context(tc.tile_pool(name="tmp", bufs=2))
        expp = ctx.enter_context(tc.tile_pool(name="expp", bufs=8))
        sbB = ctx.enter_context(tc.tile_pool(name="sbB", bufs=2))
        psA = ctx.enter_context(tc.tile_pool(name="psA", bufs=5, space="PSUM"))
        psB = ctx.enter_context(tc.tile_pool(name="psB", bufs=2, space="PSUM"))
        psD = ctx.enter_context(tc.tile_pool(name="psD", bufs=1, space="PSUM"))

        # ---- persistent tiles --------------------------------------------
        qt = const.tile([P, nH, S], bf16)    # rotated Q^T  (re rows 0-63)
        kt = const.tile([P, nH, S], bf16)    # rotated K^T
        v = const.tile([P, nST, E], bf16)    # V [s within tile, stile, e]
        outT = const.tile([P, nH, S], bf16)  # attention out^T per head
        cs_t = const.tile([P, S], bf16)      # rows 0-63 cos^T, 64-127 sin^T
        cc_t = const.tile([P, S], bf16)      # rows 0-63 AND 64-127 cos^T
        ident = const.tile([P, P], bf16)     # identity (warmup + mask-add)
        md = const.tile([P, P], bf16)        # causal: -1e9 strict lower tri
        ones_col = const.tile([P, 1], bf16)

        nc.vector.memset(ones_col, 1.0)

        # ---- warmup: identity matmuls during the initial DMA wait --------
        # (HAM un-throttles after ~3.4us of PE activity; also preload the
        # ACT exp table so the first attention chunk doesn't pay ~2.7us)
        nc.scalar.dma_start(out=ident, in_=identd)
        nc.scalar.dma_start(out=md, in_=maskd)
        wexp = tp.tile([P, 1], f32, tag="rr", name="wexp")
        nc.scalar.activation(wexp, ones_col, Exp, scale=1.0)
        warm = psB.tile([P, P], f32, tag="psB", name="warm")
        for _ in range(24):
            nc.tensor.matmul(warm, ident, ident, start=True, stop=True)

        def rope(ps, dst, col):
            """ps: [128, CW] psum raw projection (re rows 0-63, im 64-127).
            dst: [128, CW] bf16 sbuf destination slice. col: s-slice.
            5 DVE ops: PSUM inputs are exempt from the walrus shared-start-
            partition rule, so the three products each use ONE aligned SBUF
            input; the two combines are all-bf16 (2x DVE rate)."""
            rB = tp.tile([P, CW], bf16, tag="rB", name="rB")
            nc.vector.tensor_mul(rB, ps, cc_t[:, col])              # re*c|im*c
            rA = tp.tile([P, CW], bf16, tag="rA", name="rA")
            nc.vector.tensor_mul(rA[0:64], ps[64:128], cs_t[64:128, col])
            nc.vector.tensor_mul(rA[64:128], ps[0:64], cs_t[64:128, col])
            nc.vector.tensor_sub(dst[0:64], rB[0:64], rA[0:64])
            nc.vector.tensor_add(dst[64:128], rA[64:128], rB[64:128])

        # ---- attention chunk emitter -------------------------------------
        # The causal mask is accumulated INTO the scores PSUM bank by a
        # second matmul (ident stationary, -1e9-triangle moving), so
        # exp(scale*psum) is immediately the masked es.  scores+exp run
        # `ahead` iterations in front of the denominator/PV matmuls to
        # cover the cross-engine exp latency; once the prefetch well runs
        # dry (chunk tail), `filler` projection matmuls are dripped into
        # the PE stream so it never waits on ACT.
        def attn_chunk(h, c, mk=None, filler=None, ahead=4):
            qcol = slice(c * CW, (c + 1) * CW)
            jmax = TPC * c + TPC - 1 if causal else nST - 1
            ps_o = psB.tile([P, CW], f32, tag="psB", name="ps_o")
            ps_d = psD.tile([1, CW], f32, tag="psD", name="ps_d")
            ess = {}

            def drip(n):
                while filler is not None and n > 0:
                    f = next(filler, None)
                    if f is None:
                        return
                    f()
                    n -= 1

            def emit_scores(j):
                o = max(0, j - TPC * c) * P if causal else 0
                diag = causal and j >= TPC * c
                ps_s = psA.tile([P, CW], f32, tag="psA", name="ps_s")
                nc.tensor.matmul(
                    ps_s[:, o:], kt[:, h, j * P:(j + 1) * P],
                    qt[:, h, c * CW + o:(c + 1) * CW],
                    start=True, stop=not diag, skip_group_check=True)
                if diag:
                    nc.tensor.matmul(
                        ps_s[:, o:o + P], ident, md,
                        start=False, stop=True, skip_group_check=True)
                es = expp.tile([P, CW], bf16, tag="es", name="es")
                nc.scalar.activation(es[:, o:], ps_s[:, o:], Exp, scale=SCALE)
                if mask_mode == "generic":
                    nc.vector.tensor_mul(es, es, mk[:, j, :])
                ess[j] = (es, o)

            for jj in range(min(ahead, jmax + 1)):
                emit_scores(jj)
            drip(1)
            for j in range(jmax + 1):
                if j + ahead <= jmax:
                    emit_scores(j + ahead)
                else:
                    drip(2)
                es, o = ess.pop(j)
                nc.tensor.matmul(ps_d[:, o:], ones_col, es[:, o:],
                                 start=(j == 0), stop=(j == jmax))
                nc.tensor.matmul(ps_o[:, o:], v[:, j, h * P:(h + 1) * P],
                                 es[:, o:], start=(j == 0), stop=(j == jmax))
            drip(1 << 30)
            # Normalize out^T[:, sq] by 1/denom[sq].  Both PSUM banks are
            # freed by ACT copies (DVE is the busy engine in this phase);
            # broadcast / reciprocal / multiply run entirely in SBUF.
            dd = tp.tile([1, CW], f32, tag="rr", name="dd")
            nc.scalar.copy(dd, ps_d)
            ou = expp.tile([P, CW], bf16, tag="ou", name="ou", bufs=4)
            nc.scalar.copy(ou, ps_o)
            bc = sbB.tile([P, CW], f32, tag="bc", name="bc")
            nc.gpsimd.partition_broadcast(out_ap=bc, in_ap=dd)
            bcr = sbB.tile([P, CW], f32, tag="bcr", name="bcr")
            nc.vector.reciprocal_approx_fast(out=bcr, in_=bc)
            nc.vector.tensor_mul(outT[:, h, qcol], ou, bcr)

        # ---- projections + attention, engine-interleaved -----------------
        # Emission order matters: engines execute their queues in program
        # order, so attention for head h (ACT-heavy exp chain, DVE tails) is
        # emitted interleaved with head h+1's K/Q projection groups (pure PE
        # work), and the last head's attention with the wo matmul groups.
        with tc.tile_pool(name="xw", bufs=1) as xtp, \
             tc.tile_pool(name="wz", bufs=4) as wpool:
            NH2 = nDK // 2

            def load_w(key, src):
                wts = []
                for kh in range(2):
                    wt = wpool.tile([P, NH2, E], bf16, tag="w", name="wt")
                    nc.sync.dma_start(
                        out=wt, in_=src[:, kh * NH2:(kh + 1) * NH2, :])
                    wts.append(wt)
                return wts

            # DMA order: wv first (V projection runs first) on the sync
            # ring; x^T s-chunks stream on the scalar ring in parallel
            # into ONE contiguous [P, nDK, S] tile (subtile deps let the
            # V projection start as soon as chunk 0 lands).
            wv_t = load_w("v", wvr)
            xt_all = xtp.tile([P, nDK, S], bf16, tag="xt", name="xt_all")
            for kq in range(NXC):
                nc.scalar.dma_start(
                    out=xt_all[:, :, kq * SCW:(kq + 1) * SCW], in_=xr[kq])
            wk_t = load_w("k", wkr)
            nc.sync.dma_start(out=cs_t, in_=cs)
            nc.sync.dma_start(out=cc_t, in_=cc)
            wq_t = load_w("q", wqr)

            def xslice(dk, ssl):
                return xt_all[:, dk, ssl]

            def wslice(wts, dk, esl):
                return wts[dk // NH2][:, dk % NH2, esl]

            # V projection (all heads at once: rhs = all E columns);
            # s-tile st consumes x chunk st//TPC only.
            for st in range(nST):
                ssl = slice(st * P, (st + 1) * P)
                ps = psA.tile([P, CW], f32, tag="psA", name="ps_v")
                for dk in range(nDK):
                    nc.tensor.matmul(
                        ps[:, 0:E], xslice(dk, ssl), wslice(wv_t, dk, slice(0, E)),
                        start=(dk == 0), stop=(dk == nDK - 1))
                nc.scalar.copy(v[:, st, :], ps[:, 0:E])

            def qk_group(wts, dest, h, sc):
                esl = slice(h * P, (h + 1) * P)
                col = slice(sc * CW, (sc + 1) * CW)
                ps = psA.tile([P, CW], f32, tag="psA", name="ps_qk")
                for dk in range(nDK):
                    nc.tensor.matmul(
                        ps, wslice(wts, dk, esl), xslice(dk, col),
                        start=(dk == 0), stop=(dk == nDK - 1))
                rope(ps, dest[:, h, col], col)

            def qk_filler(wts, dest, h, sc):
                # qk_group as a unit-generator: each yielded callable emits
                # one PE matmul (or the trailing rope) so attn_chunk can
                # drip them into ACT-latency holes
                esl = slice(h * P, (h + 1) * P)
                col = slice(sc * CW, (sc + 1) * CW)
                box = {}

                def mm(dk):
                    if dk == 0:
                        box['ps'] = psA.tile([P, CW], f32, tag="psA",
                                             name="ps_qk")
                    nc.tensor.matmul(
                        box['ps'], wslice(wts, dk, esl), xslice(dk, col),
                        start=(dk == 0), stop=(dk == nDK - 1))

                def fin():
                    rope(box['ps'], dest[:, h, col], col)

                return iter([*(lambda d=dk: mm(d) for dk in range(nDK)), fin])

            for sc in range(nSC):
                qk_group(wk_t, kt, 0, sc)
            for sc in range(nSC):
                qk_group(wq_t, qt, 0, sc)

            if causal:
                # attention for head h is interleaved with head h+1's K/Q
                # projection groups (2 per chunk, dripped matmul-by-matmul
                # into the chunk's ACT-latency holes): the PE-only
                # projection work absorbs the exp latency of the attention
                # chain.  The last head (no projections left) runs with a
                # deeper scores prefetch instead.
                for h in range(nH):
                    if h + 1 < nH:
                        fls = ([qk_filler(wk_t, kt, h + 1, sc)
                                for sc in range(nSC)] +
                               [qk_filler(wq_t, qt, h + 1, sc)
                                for sc in range(nSC)])
                        for c in range(nSC):
                            attn_chunk(h, c,
                                       filler=chain(fls[2 * c], fls[2 * c + 1]))
                    else:
                        for c in range(nSC):
                            attn_chunk(h, c, ahead=5)
            else:
                for h in range(1, nH):
                    for sc in range(nSC):
                        qk_group(wk_t, kt, h, sc)
                        qk_group(wq_t, qt, h, sc)

        # ---- late pool (reuses xt/w space) -------------------------------
        late = ctx.enter_context(tc.tile_pool(name="late", bufs=1))
        wo_t = late.tile([P, nH, D], bf16)
        nc.sync.dma_start(out=wo_t, in_=wor)

        if not causal:
            for c in range(nSC):
                mk = None
                if mask_mode == "generic":
                    mk = late.tile([P, nST, CW], bf16, tag="mk", name="mk",
                                   bufs=2)
                    nc.sync.dma_start(
                        out=mk,
                        in_=maskT.rearrange("(j p) q -> p j q", p=P)[
                            :, :, c * CW:(c + 1) * CW])
                for h in range(nH):
                    attn_chunk(h, c, mk=mk)

        # ---- output projection (phase 3) ---------------------------------
        # Two alternating 4-bank PSUM sets so group m+1's matmuls overlap
        # group m's copies; copies split ACT/DVE; one 1 MB y-DMA per m.
        nDC = D // CW

        def wo_group(m):
            if m % 2 == 0:
                pss = [psA.tile([P, CW], f32, tag="psA", name="ps_y")
                       for _ in range(nDC)]
            else:
                pss = [psB.tile([P, CW], f32, tag="psB", name="ps_y"),
                       psB.tile([P, CW], f32, tag="psB", name="ps_y"),
                       psD.tile([P, CW], f32, tag="psD", name="ps_y"),
                       psA.tile([P, CW], f32, tag="psA", name="ps_y")]
            for h in range(nH):
                for dc in range(nDC):
                    nc.tensor.matmul(
                        pss[dc], outT[:, h, m * P:(m + 1) * P],
                        wo_t[:, h, dc * CW:(dc + 1) * CW],
                        start=(h == 0), stop=(h == nH - 1))
            yo = late.tile([P, D], bf16, tag="yo", name="yo", bufs=2)
            split = m >= nST - 2   # shorten the kernel tail
            for dc in range(nDC):
                dst = yo[:, dc * CW:(dc + 1) * CW]
                if dc % 2 == 0:
                    nc.scalar.copy(dst, pss[dc])
                else:
                    nc.vector.tensor_copy(dst, pss[dc])
                if split:
                    nc.sync.dma_start(
                        out=y[m * P:(m + 1) * P, dc * CW:(dc + 1) * CW],
                        in_=dst)
            if not split:
                nc.sync.dma_start(out=y[m * P:(m + 1) * P, :], in_=yo)

        for m in range(nST):
            wo_group(m)

    nc.compile()
    return nc


def _get_built(mask_mode, S, D, E):
    key = (mask_mode, S, D, E)
    if key not in _built_cache:
        _built_cache[key] = _build(S=S, D=D, E=E, mask_mode=mask_mode)
    return _built_cache[key]


def _classify_mask(mask):
    S = mask.shape[0]
    if not mask.any():
        return "none"
    causal = np.where(np.triu(np.ones((S, S), dtype=bool), k=1),
                      np.float32(-1e9), np.float32(0.0))
    if np.array_equal(mask, causal):
        return "causal"
    return "generic"


def make_in_maps(x, wq, wk, wv, wo, freqs_cos, freqs_sin, mask, n_cores=8):
    """Host-side sharding + layout prep. Returns (in_maps, mask_mode, meta)."""
    bf = ml_dtypes.bfloat16
    x = np.asarray(x, np.float32)
    B, S, D = x.shape
    groups = n_cores // B
    E = D // groups
    nH = E // P
    nDK = D // P

    mask = np.asarray(mask, np.float32)
    mode = _classify_mask(mask)

    fc = np.asarray(freqs_cos, np.float32)
    fs = np.asarray(freqs_sin, np.float32)
    cs = np.concatenate(
        [np.ascontiguousarray(fc.T), np.ascontiguousarray(fs.T)], axis=0
    ).astype(bf)                              # [128, S]: [cos; sin]
    cc = np.concatenate(
        [np.ascontiguousarray(fc.T), np.ascontiguousarray(fc.T)], axis=0
    ).astype(bf)                              # [128, S]: [cos; cos]

    identd = np.eye(P, dtype=np.float32).astype(bf)
    if mode == "causal":
        # strict lower triangle (sk > sq) gets -1e9: added into scores PSUM
        maskd = np.where(np.arange(P)[:, None] > np.arange(P)[None, :],
                         np.float32(-1e9), np.float32(0.0)).astype(bf)
    else:
        maskd = np.zeros((P, P), np.float32).astype(bf)

    # per-head deinterleave: head-local columns [0,2,...,126,1,3,...,127]
    perm1 = np.concatenate([np.arange(0, P, 2), np.arange(1, P, 2)])
    permE = np.concatenate([h * P + perm1 for h in range(nH)])

    wqT_f = np.asarray(wq, np.float32).T      # [D, D]
    wkT_f = np.asarray(wk, np.float32).T
    wvT_f = np.asarray(wv, np.float32).T
    woT_f = np.asarray(wo, np.float32).T      # [E_total, D]

    if mode == "generic":
        maskT_bf = np.exp(np.ascontiguousarray(mask.T)).astype(bf)

    def swz_w(wT):   # [D, E] -> [P, nDK, E]
        return np.ascontiguousarray(
            wT.reshape(nDK, P, E).transpose(1, 0, 2)).astype(bf)

    SCW = S // NXC

    def swz_x(xT):   # [D, S] -> [NXC, P, nDK, SCW]
        return np.ascontiguousarray(
            xT.reshape(nDK, P, NXC, SCW).transpose(2, 1, 0, 3)).astype(bf)

    xr_b = [swz_x(x[b].T) for b in range(B)]

    in_maps = []
    for c in range(n_cores):
        b, g = divmod(c, groups)
        es = slice(g * E, (g + 1) * E)
        woT_g = woT_f[es, :]                  # [E, D]
        m = {
            "xr": xr_b[b],
            "wqr": swz_w(wqT_f[:, es][:, permE]),
            "wkr": swz_w(wkT_f[:, es][:, permE]),
            "wvr": swz_w(wvT_f[:, es]),
            "wor": np.ascontiguousarray(
                woT_g.reshape(nH, P, D).transpose(1, 0, 2)).astype(bf),
            "cs": cs,
            "cc": cc,
            "identd": identd,
            "maskd": maskd,
        }
        if mode == "generic":
            m["maskT"] = maskT_bf
        in_maps.append(m)
    return in_maps, mode, (B, S, D, E, groups)


def kernel(x, wq, wk, wv, wo, freqs_cos, freqs_sin, mask, start_pos=0, **_):
    from concourse.bass_utils import run_bass_kernel_spmd

    in_maps, mode, (B, S, D, E, groups) = make_in_maps(
        x, wq, wk, wv, wo, freqs_cos, freqs_sin, mask)
    nc = _get_built(mode, S, D, E)
    res = run_bass_kernel_spmd(nc, in_maps, core_ids=list(range(len(in_maps))))
    parts = [np.asarray(r["y"], np.float32) for r in res.results]
    out = np.stack(
        [np.sum(parts[b * groups:(b + 1) * groups], axis=0) for b in range(B)]
    ).astype(np.float32)
    return out


# revision 18
# speedup vs baseline: 1.1115x; 1.0098x over previous
"""Trainium2 Bass kernel: multi-head causal attention with RoPE (LLaMA-style).

Problem: y = Attention(x) with B=2, S=2048, D=2048, H=16 heads, HD=128,
torch-Linear convention (y = x @ W.T), interleaved-rope, additive mask.

Sharding (8 NeuronCores): batch (2) x head-groups (4) grid.  Core c handles
batch b = c // 4 and heads 4g..4g+3 where g = c % 4 (tensor parallel:
wq/wk/wv column-parallel, wo row-parallel).  Each core returns a partial
y contribution [S, D]; the host sums the 4 partials per batch.

Layout strategy (no on-chip transposes anywhere):
  - Host pre-transposes AND pre-swizzles DMA layouts: x as 4 s-major chunks
    [P, nDK, 512] (so the V projection can start after 1/4 of x lands),
    wq/wk/wv as [P, nDK, E], wo as [P, nH, D] -- every DMA is a contiguous
    per-partition run.
  - Q^T,K^T computed directly in [hd, s] layout (hd = partitions) with the
    head-dim DEINTERLEAVED (rows 0-63 = even/"re" dims, 64-127 = odd/"im")
    by permuting wq/wk columns on the host; RoPE is then 5 DVE ops per
    chunk (products exploit the walrus PSUM-input exemption; combines run
    all-bf16 at 2x DVE rate).
  - scores are computed TRANSPOSED [sk, sq] so softmax-denominators come
    from a ones-matmul (column sums) and exp(scores)^T feeds the PV matmul
    directly as the moving operand: P^T never materializes.
  - causal mask applied INSIDE PSUM by an extra accumulating matmul
    (identity stationary x (-1e9 triangle) moving) so exp sees masked
    scores directly -- no DVE op or cross-engine hop on the es path.
  - attention out falls out as out^T [hd, sq] = exactly the stationary
    layout the wo row-parallel matmul wants.  wo phase double-buffers two
    4-bank PSUM sets, splits PSUM->SBUF copies between ACT and DVE, and
    ships y as 1 MB per-row-tile DMAs.
  - PE is warmed with identity matmuls during the initial DMA wait (HAM
    clock-gate ramps after ~3.4us of activity).
Matmul inputs are bf16 (fp32 PSUM accumulation); softmax runs in fp32.
"""

import math
from contextlib import ExitStack
from itertools import chain

import numpy as np
import ml_dtypes

P = 128          # partitions / head dim
CW = 512         # s-chunk width (one PSUM bank of fp32)
NXC = 4          # x DMA chunks along s

_built_cache = {}


def _build(*, S, D, E, mask_mode):
    """Build + compile the SPMD Bass program for one core's shard.

    S: sequence length, D: model dim, E: head-columns per core (nH*128).
    mask_mode: 'causal' (use diag mask-add matmul + skip upper triangle),
               'none' (no mask, full attention),
               'generic' (arbitrary additive mask, applied everywhere).
    """
    import concourse.bacc as bacc
    import concourse.mybir as mybir
    import concourse.tile as tile

    f32 = mybir.dt.float32
    bf16 = mybir.dt.bfloat16
    Exp = mybir.ActivationFunctionType.Exp

    nDK = D // P       # k-tiles over model dim
    nH = E // P        # heads on this core
    nSC = S // CW      # 512-wide s-chunks
    nST = S // P       # 128-wide s-tiles
    TPC = CW // P      # s-tiles per chunk (4)
    SCW = S // NXC     # x chunk width (512)
    SCALE = 1.0 / math.sqrt(P)
    causal = mask_mode == "causal"

    nc = bacc.Bacc("TRN2", target_bir_lowering=False, debug=False)

    # host pre-swizzled layouts (all contiguous per-partition runs)
    xr = nc.dram_tensor("xr", [NXC, P, nDK, SCW], bf16, kind="ExternalInput").ap()
    wqr = nc.dram_tensor("wqr", [P, nDK, E], bf16, kind="ExternalInput").ap()
    wkr = nc.dram_tensor("wkr", [P, nDK, E], bf16, kind="ExternalInput").ap()
    wvr = nc.dram_tensor("wvr", [P, nDK, E], bf16, kind="ExternalInput").ap()
    wor = nc.dram_tensor("wor", [P, nH, D], bf16, kind="ExternalInput").ap()
    cs = nc.dram_tensor("cs", [P, S], bf16, kind="ExternalInput").ap()   # [cos;sin]
    cc = nc.dram_tensor("cc", [P, S], bf16, kind="ExternalInput").ap()   # [cos;cos]
    identd = nc.dram_tensor("identd", [P, P], bf16, kind="ExternalInput").ap()
    maskd = nc.dram_tensor("maskd", [P, P], bf16, kind="ExternalInput").ap()
    if mask_mode == "generic":
        maskT = nc.dram_tensor("maskT", [S, S], bf16, kind="ExternalInput").ap()
    y = nc.dram_tensor("y", [S, D], bf16, kind="ExternalOutput").ap()

    with tile.TileContext(nc) as tc, ExitStack() as ctx:
        const = ctx.enter_context(tc.tile_pool(name="const", bufs=1))
        tp = ctx.enter_context(tc.tile_pool(name="tmp", bufs=2))
        expp = ctx.enter_context(tc.tile_pool(name="expp", bufs=8))
        sbB = ctx.enter_context(tc.tile_pool(name="sbB", bufs=2))
        psA = ctx.enter_context(tc.tile_pool(name="psA", bufs=5, space="PSUM"))
        psB = ctx.enter_context(tc.tile_pool(name="psB", bufs=2, space="PSUM"))
        psD = ctx.enter_context(tc.tile_pool(name="psD", bufs=1, space="PSUM"))

        # ---- persistent tiles --------------------------------------------
        qt = const.tile([P, nH, S], bf16)    # rotated Q^T  (re rows 0-63)
        kt = const.tile([P, nH, S], bf16)    # rotated K^T
        v = const.tile([P, nST, E], bf16)    # V [s within tile, stile, e]
        outT = const.tile([P, nH, S], bf16)  # attention out^T per head
        cs_t = const.tile([P, S], bf16)      # rows 0-63 cos^T, 64-127 sin^T
        cc_t = const.tile([P, S], bf16)      # rows 0-63 AND 64-127 cos^T
        ident = const.tile([P, P], bf16)     # identity (warmup + mask-add)
        md = const.tile([P, P], bf16)        # causal: -1e9 strict lower tri
        ones_col = const.tile([P, 1], bf16)

        nc.vector.memset(ones_col, 1.0)

        # ---- warmup: identity matmuls during the initial DMA wait --------
        # (HAM un-throttles after ~3.4us of PE activity; also preload the
        # ACT exp table so the first attention chunk doesn't pay ~2.7us)
        nc.scalar.dma_start(out=ident, in_=identd)
        nc.scalar.dma_start(out=md, in_=maskd)
        wexp = tp.tile([P, 1], f32, tag="rr", name="wexp")
        nc.scalar.activation(wexp, ones_col, Exp, scale=1.0)
        warm = psB.tile([P, P], f32, tag="psB", name="warm")
        for _ in range(36):
            nc.tensor.matmul(warm, ident, ident, start=True, stop=True)

        def rope(ps, dst, col):
            """ps: [128, CW] psum raw projection (re rows 0-63, im 64-127).
            dst: [128, CW] bf16 sbuf destination slice. col: s-slice.
            5 DVE ops: PSUM inputs are exempt from the walrus shared-start-
            partition rule, so the three products each use ONE aligned SBUF
            input; the two combines are all-bf16 (2x DVE rate)."""
            rB = tp.tile([P, CW], bf16, tag="rB", name="rB")
            nc.vector.tensor_mul(rB, ps, cc_t[:, col])              # re*c|im*c
            rA = tp.tile([P, CW], bf16, tag="rA", name="rA")
            nc.vector.tensor_mul(rA[0:64], ps[64:128], cs_t[64:128, col])
            nc.vector.tensor_mul(rA[64:128], ps[0:64], cs_t[64:128, col])
            nc.vector.tensor_sub(dst[0:64], rB[0:64], rA[0:64])
            nc.vector.tensor_add(dst[64:128], rA[64:128], rB[64:128])

        # ---- attention chunk emitter -------------------------------------
        # The causal mask is accumulated INTO the scores PSUM bank by a
        # second matmul (ident stationary, -1e9-triangle moving), so
        # exp(scale*psum) is immediately the masked es.  scores+exp run
        # `ahead` iterations in front of the denominator/PV matmuls to
        # cover the cross-engine exp latency; once the prefetch well runs
        # dry (chunk tail), `filler` projection matmuls are dripped into
        # the PE stream so it never waits on ACT.
        def attn_chunk(h, c, mk=None, filler=None, ahead=4):
            qcol = slice(c * CW, (c + 1) * CW)
            jmax = TPC * c + TPC - 1 if causal else nST - 1
            ps_o = psB.tile([P, CW], f32, tag="psB", name="ps_o")
            ps_d = psD.tile([1, CW], f32, tag="psD", name="ps_d")
            ess = {}

            def drip(n):
                while filler is not None and n > 0:
                    f = next(filler, None)
                    if f is None:
                        return
                    f()
                    n -= 1

            def emit_scores(j):
                o = max(0, j - TPC * c) * P if causal else 0
                diag = causal and j >= TPC * c
                ps_s = psA.tile([P, CW], f32, tag="psA", name="ps_s")
                nc.tensor.matmul(
                    ps_s[:, o:], kt[:, h, j * P:(j + 1) * P],
                    qt[:, h, c * CW + o:(c + 1) * CW],
                    start=True, stop=not diag, skip_group_check=True)
                if diag:
                    nc.tensor.matmul(
                        ps_s[:, o:o + P], ident, md,
                        start=False, stop=True, skip_group_check=True)
                es = expp.tile([P, CW], bf16, tag="es", name="es")
                nc.scalar.activation(es[:, o:], ps_s[:, o:], Exp, scale=SCALE)
                if mask_mode == "generic":
                    nc.vector.tensor_mul(es, es, mk[:, j, :])
                ess[j] = (es, o)

            for jj in range(min(ahead, jmax + 1)):
                emit_scores(jj)
            drip(1)
            for j in range(jmax + 1):
                if j + ahead <= jmax:
                    emit_scores(j + ahead)
                else:
                    drip(2)
                es, o = ess.pop(j)
                nc.tensor.matmul(ps_d[:, o:], ones_col, es[:, o:],
                                 start=(j == 0), stop=(j == jmax))
                nc.tensor.matmul(ps_o[:, o:], v[:, j, h * P:(h + 1) * P],
                                 es[:, o:], start=(j == 0), stop=(j == jmax))
            drip(1 << 30)
            # Normalize out^T[:, sq] by 1/denom[sq].  Both PSUM banks are
            # freed by ACT copies (DVE is the busy engine in this phase);
            # broadcast / reciprocal / multiply run entirely in SBUF.
            dd = tp.tile([1, CW], f32, tag="rr", name="dd")
            nc.scalar.copy(dd, ps_d)
            ou = expp.tile([P, CW], bf16, tag="ou", name="ou", bufs=4)
            nc.scalar.copy(ou, ps_o)
            bc = sbB.tile([P, CW], f32, tag="bc", name="bc")
            nc.gpsimd.partition_broadcast(out_ap=bc, in_ap=dd)
            bcr = sbB.tile([P, CW], f32, tag="bcr", name="bcr")
            nc.vector.reciprocal_approx_fast(out=bcr, in_=bc)
            nc.vector.tensor_mul(outT[:, h, qcol], ou, bcr)

        # ---- projections + attention, engine-interleaved -----------------
        # Emission order matters: engines execute their queues in program
        # order, so attention for head h (ACT-heavy exp chain, DVE tails) is
        # emitted interleaved with head h+1's K/Q projection groups (pure PE
        # work), and the last head's attention with the wo matmul groups.
        with tc.tile_pool(name="xw", bufs=1) as xtp, \
             tc.tile_pool(name="wz", bufs=4) as wpool:
            NH2 = nDK // 2

            def load_w(key, src):
                wts = []
                for kh in range(2):
                    wt = wpool.tile([P, NH2, E], bf16, tag="w", name="wt")
                    nc.sync.dma_start(
                        out=wt, in_=src[:, kh * NH2:(kh + 1) * NH2, :])
                    wts.append(wt)
                return wts

            # DMA order: wv first (V projection runs first) on the sync
            # ring; x^T s-chunks stream on the scalar ring in parallel
            # into ONE contiguous [P, nDK, S] tile (subtile deps let the
            # V projection start as soon as chunk 0 lands).
            wv_t = load_w("v", wvr)
            xts = []
            for kq in range(NXC):
                xt = xtp.tile([P, nDK, SCW], bf16, tag=f"xt{kq}", name="xt")
                nc.scalar.dma_start(out=xt, in_=xr[kq])
                xts.append(xt)
            wk_t = load_w("k", wkr)
            nc.sync.dma_start(out=cs_t, in_=cs)
            nc.sync.dma_start(out=cc_t, in_=cc)
            wq_t = load_w("q", wqr)

            def xslice(dk, ssl):
                lo, hi = ssl.start, ssl.stop
                return xts[lo // SCW][:, dk, lo % SCW:lo % SCW + (hi - lo)]

            def wslice(wts, dk, esl):
                return wts[dk // NH2][:, dk % NH2, esl]

            # V projection (all heads at once: rhs = all E columns);
            # s-tile st consumes x chunk st//TPC only.
            for st in range(nST):
                ssl = slice(st * P, (st + 1) * P)
                ps = psA.tile([P, CW], f32, tag="psA", name="ps_v")
                for dk in range(nDK):
                    nc.tensor.matmul(
                        ps[:, 0:E], xslice(dk, ssl), wslice(wv_t, dk, slice(0, E)),
                        start=(dk == 0), stop=(dk == nDK - 1))
                nc.scalar.copy(v[:, st, :], ps[:, 0:E])

            def qk_group(wts, dest, h, sc):
                esl = slice(h * P, (h + 1) * P)
                col = slice(sc * CW, (sc + 1) * CW)
                ps = psA.tile([P, CW], f32, tag="psA", name="ps_qk")
                for dk in range(nDK):
                    nc.tensor.matmul(
                        ps, wslice(wts, dk, esl), xslice(dk, col),
                        start=(dk == 0), stop=(dk == nDK - 1))
                rope(ps, dest[:, h, col], col)

            def qk_filler(wts, dest, h, sc):
                # qk_group as a unit-generator: each yielded callable emits
                # one PE matmul (or the trailing rope) so attn_chunk can
                # drip them into ACT-latency holes
                esl = slice(h * P, (h + 1) * P)
                col = slice(sc * CW, (sc + 1) * CW)
                box = {}

                def mm(dk):
                    if dk == 0:
                        box['ps'] = psA.tile([P, CW], f32, tag="psA",
                                             name="ps_qk")
                    nc.tensor.matmul(
                        box['ps'], wslice(wts, dk, esl), xslice(dk, col),
                        start=(dk == 0), stop=(dk == nDK - 1))

                def fin():
                    rope(box['ps'], dest[:, h, col], col)

                return iter([*(lambda d=dk: mm(d) for dk in range(nDK)), fin])

            for sc in range(nSC):
                qk_group(wk_t, kt, 0, sc)
            for sc in range(nSC):
                qk_group(wq_t, qt, 0, sc)

            if causal:
                # attention for head h is interleaved with head h+1's K/Q
                # projection groups (2 per chunk, dripped matmul-by-matmul
                # into the chunk's ACT-latency holes): the PE-only
                # projection work absorbs the exp latency of the attention
                # chain.  The last head (no projections left) runs with a
                # deeper scores prefetch instead.
                for h in range(nH):
                    if h + 1 < nH:
                        fls = ([qk_filler(wk_t, kt, h + 1, sc)
                                for sc in range(nSC)] +
                               [qk_filler(wq_t, qt, h + 1, sc)
                                for sc in range(nSC)])
                        for c in range(nSC):
                            attn_chunk(h, c,
                                       filler=chain(fls[2 * c], fls[2 * c + 1]))
                    else:
                        for c in range(nSC):
                            attn_chunk(h, c, ahead=5)
            else:
                for h in range(1, nH):
                    for sc in range(nSC):
                        qk_group(wk_t, kt, h, sc)
                        qk_group(wq_t, qt, h, sc)

        # ---- late pool (reuses xt/w space) -------------------------------
        late = ctx.enter_context(tc.tile_pool(name="late", bufs=1))
        wo_t = late.tile([P, nH, D], bf16)
        nc.sync.dma_start(out=wo_t, in_=wor)

        if not causal:
            for c in range(nSC):
                mk = None
                if mask_mode == "generic":
                    mk = late.tile([P, nST, CW], bf16, tag="mk", name="mk",
                                   bufs=2)
                    nc.sync.dma_start(
                        out=mk,
                        in_=maskT.rearrange("(j p) q -> p j q", p=P)[
                            :, :, c * CW:(c + 1) * CW])
                for h in range(nH):
                    attn_chunk(h, c, mk=mk)

        # ---- output projection (phase 3) ---------------------------------
        # Two alternating 4-bank PSUM sets so group m+1's matmuls overlap
        # group m's copies; copies split ACT/DVE; one 1 MB y-DMA per m.
        nDC = D // CW

        def wo_group(m):
            if m % 2 == 0:
                pss = [psA.tile([P, CW], f32, tag="psA", name="ps_y")
                       for _ in range(nDC)]
            else:
                pss = [psB.tile([P, CW], f32, tag="psB", name="ps_y"),
                       psB.tile([P, CW], f32, tag="psB", name="ps_y"),
                       psD.tile([P, CW], f32, tag="psD", name="ps_y"),
                       psA.tile([P, CW], f32, tag="psA", name="ps_y")]
            for h in range(nH):
                for dc in range(nDC):
                    nc.tensor.matmul(
                        pss[dc], outT[:, h, m * P:(m + 1) * P],
                        wo_t[:, h, dc * CW:(dc + 1) * CW],
                        start=(h == 0), stop=(h == nH - 1))
            yo = late.tile([P, D], bf16, tag="yo", name="yo", bufs=2)
            split = m >= nST - 2   # shorten the kernel tail
            for dc in range(nDC):
                dst = yo[:, dc * CW:(dc + 1) * CW]
                if dc % 2 == 0:
                    nc.scalar.copy(dst, pss[dc])
                else:
                    nc.vector.tensor_copy(dst, pss[dc])
                if split:
                    nc.sync.dma_start(
                        out=y[m * P:(m + 1) * P, dc * CW:(dc + 1) * CW],
                        in_=dst)
            if not split:
                nc.sync.dma_start(out=y[m * P:(m + 1) * P, :], in_=yo)

        for m in range(nST):
            wo_group(m)

    nc.compile()
    return nc


def _get_built(mask_mode, S, D, E):
    key = (mask_mode, S, D, E)
    if key not in _built_cache:
        _built_cache[key] = _build(S=S, D=D, E=E, mask_mode=mask_mode)
    return _built_cache[key]


def _classify_mask(mask):
    S = mask.shape[0]
    if not mask.any():
        return "none"
    causal = np.where(np.triu(np.ones((S, S), dtype=bool), k=1),
                      np.float32(-1e9), np.float32(0.0))
    if np.array_equal(mask, causal):
        return "causal"
    return "generic"


def make_in_maps(x, wq, wk, wv, wo, freqs_cos, freqs_sin, mask, n_cores=8):
    """Host-side sharding + layout prep. Returns (in_maps, mask_mode, meta)."""
    bf = ml_dtypes.bfloat16
    x = np.asarray(x, np.float32)
    B, S, D = x.shape
    groups = n_cores // B
    E = D // groups
    nH = E // P
    nDK = D // P

    mask = np.asarray(mask, np.float32)
    mode = _classify_mask(mask)

    fc = np.asarray(freqs_cos, np.float32)
    fs = np.asarray(freqs_sin, np.float32)
    cs = np.concatenate(
        [np.ascontiguousarray(fc.T), np.ascontiguousarray(fs.T)], axis=0
    ).astype(bf)                              # [128, S]: [cos; sin]
    cc = np.concatenate(
        [np.ascontiguousarray(fc.T), np.ascontiguousarray(fc.T)], axis=0
    ).astype(bf)                              # [128, S]: [cos; cos]

    identd = np.eye(P, dtype=np.float32).astype(bf)
    if mode == "causal":
        # strict lower triangle (sk > sq) gets -1e9: added into scores PSUM
        maskd = np.where(np.arange(P)[:, None] > np.arange(P)[None, :],
                         np.float32(-1e9), np.float32(0.0)).astype(bf)
    else:
        maskd = np.zeros((P, P), np.float32).astype(bf)

    # per-head deinterleave: head-local columns [0,2,...,126,1,3,...,127]
    perm1 = np.concatenate([np.arange(0, P, 2), np.arange(1, P, 2)])
    permE = np.concatenate([h * P + perm1 for h in range(nH)])

    wqT_f = np.asarray(wq, np.float32).T      # [D, D]
    wkT_f = np.asarray(wk, np.float32).T
    wvT_f = np.asarray(wv, np.float32).T
    woT_f = np.asarray(wo, np.float32).T      # [E_total, D]

    if mode == "generic":
        maskT_bf = np.exp(np.ascontiguousarray(mask.T)).astype(bf)

    def swz_w(wT):   # [D, E] -> [P, nDK, E]
        return np.ascontiguousarray(
            wT.reshape(nDK, P, E).transpose(1, 0, 2)).astype(bf)

    SCW = S // NXC

    def swz_x(xT):   # [D, S] -> [NXC, P, nDK, SCW]
        return np.ascontiguousarray(
            xT.reshape(nDK, P, NXC, SCW).transpose(2, 1, 0, 3)).astype(bf)

    xr_b = [swz_x(x[b].T) for b in range(B)]

    in_maps = []
    for c in range(n_cores):
        b, g = divmod(c, groups)
        es = slice(g * E, (g + 1) * E)
        woT_g = woT_f[es, :]                  # [E, D]
        m = {
            "xr": xr_b[b],
            "wqr": swz_w(wqT_f[:, es][:, permE]),
            "wkr": swz_w(wkT_f[:, es][:, permE]),
            "wvr": swz_w(wvT_f[:, es]),
            "wor": np.ascontiguousarray(
                woT_g.reshape(nH, P, D).transpose(1, 0, 2)).astype(bf),
            "cs": cs,
            "cc": cc,
            "identd": identd,
            "maskd": maskd,
        }
        if mode == "generic":
            m["maskT"] = maskT_bf
        in_maps.append(m)
    return in_maps, mode, (B, S, D, E, groups)


def kernel(x, wq, wk, wv, wo, freqs_cos, freqs_sin, mask, start_pos=0, **_):
    from concourse.bass_utils import run_bass_kernel_spmd

    in_maps, mode, (B, S, D, E, groups) = make_in_maps(
        x, wq, wk, wv, wo, freqs_cos, freqs_sin, mask)
    nc = _get_built(mode, S, D, E)
    res = run_bass_kernel_spmd(nc, in_maps, core_ids=list(range(len(in_maps))))
    parts = [np.asarray(r["y"], np.float32) for r in res.results]
    out = np.stack(
        [np.sum(parts[b * groups:(b + 1) * groups], axis=0) for b in range(B)]
    ).astype(np.float32)
    return out
